# revision 32
# baseline (speedup 1.0000x reference)
"""Position-attention (SAGAN-style) Bass kernel for 8 Trainium2 NeuronCores.

Reference computation (per batch b, with n = H*W = 4096 spatial tokens):
    q = Wq @ x + bq            [32, n]
    k = Wk @ x + bk            [32, n]
    v = Wv @ x + bv            [256, n]
    att = softmax_j(q_i . k_j) [n, n]
    out = gamma * (v @ att^T) + x

Sharding: 8 cores = 4 batches x 2 token-halves. Each core computes out for
its 2048-token half (the q/i side); k/v cover all 4096 tokens redundantly
(cheap). Output slices are disjoint -> no collectives. To keep one SPMD
program, the host rotates each core's x so its own half occupies columns
0:2048 (j-order is irrelevant to the softmax sum; k/v/e stay consistent).

Engine/schedule choices (all aimed at keeping the PE array dense):
  - x and the weights are pre-rounded to bf16 on the host and DMAed in a
    per-partition-contiguous layout; q/k projections interleave with the
    chunked x transfers so the PE starts before the load completes.
  - scores^T[j, i] (K=32) as 2x row-packed pairs into bank-aligned PSUM
    slices; consecutive pairs alternate PE row-tile quadrants
    (T0/T4 <-> T8/T12) so back-to-back pairs overlap (measured ~2x); exp
    on ACT reads each pair tile.
  - rowsum[i] = sum_j e^T[j, i] via column-tiled M=32 matmuls (partial
    sums replicated in partitions 32t:32t+32, accumulated over j-quads,
    measured ~4x concurrency). Four column-mode zero matmuls claim the
    bank first (matmul start=True invalidates has_written for the whole
    bank, so the four interleaved chains must accumulate with
    start=False onto an explicitly zero-filled bank). One fp32 matmul
    (lhsT = ones/32) then fuses combine + broadcast-to-128-partitions;
    reciprocal (DVE) and gamma give the per-i scale.
  - software pipeline per i-block b: scores(b) | out-chains(b-1) |
    rowsum(b) -- exp(b) hides under out(b-1); v-projection slots between
    scores(b0) and rowsum(b0) to fill the pipeline prologue.
  - out[c, i]: K=128 per j-tile, 32-matmul PSUM accumulation chains per
    (i-block, c-half); epilogue folds gamma/rowsum, gamma*bv and the
    bf16-rounded residual + x, split in halves to overlap the store DMA.
Matmul operands are bf16 (fp32 PSUM accumulation). PSUM budget: psA
3 bufs x 2 banks (score pairs, rowsum accumulator rides a psA slot) +
psB 2 bufs x 1 bank (v-proj/out chains, combine/broadcast) = 8 banks.
"""

import os
import sys

for _p in (
    "/root/.axon_site",
    "/root/.axon_site/_ro/trn_rl_repo",
    "/root/.axon_site/_ro/pypackages",
    "/opt/trn_rl_repo",
):
    if os.path.isdir(_p) and _p not in sys.path:
        sys.path.append(_p)

import json

import numpy as np

from concourse import bass, mybir
from concourse.tile import TileContext

F32 = mybir.dt.float32
BF16 = mybir.dt.bfloat16

B, C, H, W = 4, 256, 64, 64
N = H * W            # 4096 tokens
NH = N // 2          # 2048 tokens per core (token half)
MID = C // 8         # 32 qk channels
JT = N // 128        # 32 j-tiles of 128 tokens
NBLK = NH // 512     # 4 i-blocks of 512 tokens per core
NQUAD = JT // 4      # 8 quads of 4 j-tiles


def _split_multi_waits(bir_bytes: bytes) -> bytes:
    """Workaround for this container's walrus: it accepts at most ONE sem-wait
    command per lowered instruction ('Too many sync wait commands'), while
    bass/Tile freely attach several. Split extra waits onto preceding NoOps
    on the same engine — per-engine program order makes this semantics-
    preserving (all waits still satisfied before the instruction runs)."""
    d = json.loads(bir_bytes)
    n_split = 0
    for f in d.get("functions", []):
        for bb in f.get("blocks", []):
            out = []
            for ins in bb.get("instructions", []):
                si = ins.get("sync_info")
                waits = si.get("on_wait") if si else None
                if waits and len(waits) > 1:
                    for w in waits[:-1]:
                        n_split += 1
                        out.append(
                            {
                                "debug": ins.get("debug", 0),
                                "engine": ins["engine"],
                                "ins": [],
                                "outs": [],
                                "name": f"{ins['name']}-ws{n_split}",
                                "opcode": "NoOp",
                                "sync_info": {"on_wait": [w], "on_update": []},
                            }
                        )
                    si["on_wait"] = [waits[-1]]
                out.append(ins)
            bb["instructions"] = out
    return json.dumps(d).encode()


_ws_applied = False


def _apply_wait_split_patch():
    global _ws_applied
    if _ws_applied:
        return
    _ws_applied = True
    from concourse import bass_utils, bass2jax

    orig = bass_utils.compile_bir_kernel

    def patched(bir_json, tmpdir, neff_name="file.neff"):
        return orig(_split_multi_waits(bytes(bir_json)), tmpdir, neff_name)

    bass_utils.compile_bir_kernel = patched
    bass2jax.compile_bir_kernel = patched


_apply_wait_split_patch()


def _build_program():
    nc = bass.Bass()

    xb_d = nc.declare_dram_parameter("xb", [128, 2 * N], BF16, isOutput=False)
    wT_d = nc.declare_dram_parameter("wT", [C, 512], BF16, isOutput=False)
    cst_d = nc.declare_dram_parameter("cst", [128, 5], F32, isOutput=False)
    out_d = nc.declare_dram_parameter("out", [C, NH], F32, isOutput=True)

    act = mybir.ActivationFunctionType

    with TileContext(nc) as tc:
        with (
            tc.tile_pool(name="const", bufs=1) as constp,
            tc.tile_pool(name="xb", bufs=1) as xbp,
            tc.tile_pool(name="proj", bufs=1) as projp,
            tc.tile_pool(name="eblk", bufs=2) as eblkp,
            tc.tile_pool(name="small", bufs=4) as smallp,
            tc.tile_pool(name="res", bufs=4) as resp,
            tc.tile_pool(name="psA", bufs=3, space="PSUM") as psA,
            tc.tile_pool(name="psB", bufs=2, space="PSUM") as psB,
        ):
            # ---- constants / weights ----
            w_b = constp.tile([128, 2, 512], BF16, tag="wb")
            for h in range(2):
                nc.sync.dma_start(out=w_b[:, h, :],
                                  in_=wT_d[h * 128:(h + 1) * 128, :])

            cst = constp.tile([128, 5], F32, tag="cst")
            nc.sync.dma_start(out=cst[:, :], in_=cst_d[:, :])
            bq4 = cst[:, 0:1]
            bk4 = cst[:, 1:2]
            bvP = cst[:, 2:4]
            g128 = cst[:, 4:5]
            # gb[c] = gamma * bv[c]  (folded v-bias: out += gamma*bv[c])
            gb = constp.tile([128, 2], F32, tag="gb")
            nc.vector.tensor_scalar_mul(gb[:, :], bvP[:, :], g128[:, :])

            # ones[128, 32] bf16 for the column-tiled rowsum matmuls
            ones_rs = constp.tile([128, 32], BF16, tag="ones_rs")
            nc.vector.memset(ones_rs[:, :], 1.0)
            # zeros for the column-mode matmuls that claim the rowsum bank
            zeros_rs = constp.tile([128, 32], BF16, tag="zeros_rs")
            nc.vector.memset(zeros_rs[:, :], 0.0)
            # combine+broadcast weights: rowsum lands 32x-replicated per
            # column group, so sum over all 128 partitions / 32 = rowsum
            sel_w = constp.tile([128, 128], F32, tag="sel_w")
            nc.vector.memset(sel_w[:, :], 1.0 / 32.0)

            # HAM warm-up: ~4us of dummy matmuls while the x DMA is in
            # flight, so the PE clock-gate is already at 8/8 (2.4 GHz)
            # when the first projection matmuls issue. Results are never
            # read; the psA slot is recycled by the scores pipeline.
            warm_rhs = constp.tile([128, 512], BF16, tag="warm_rhs")
            nc.vector.memset(warm_rhs[:, :], 0.0)
            warm_ps = psA.tile([128, 512], F32, tag="psa", name="warm_ps")
            for _ in range(10):
                nc.tensor.matmul(warm_ps[0:32, :], lhsT=zeros_rs[:, :],
                                 rhs=warm_rhs[:, :], start=True, stop=True,
                                 skip_group_check=True)

            # ---- x load (chunked, SP ring, bf16) + q/k projections ----
            # host layout [128, c8, h, 512]: per-partition contiguous so
            # each chunk DMA is one descriptor per partition
            x_b = xbp.tile([128, 8, 2, 512], BF16, tag="xb")
            q_sb = projp.tile([128, NH], BF16, tag="q")
            k_sb = projp.tile([128, N], BF16, tag="k")
            v_sb = projp.tile([128, JT, C], BF16, tag="v")

            for c8 in range(8):
                sl = slice(c8 * 512, (c8 + 1) * 512)
                if c8 % 2 == 0:
                    nc.sync.dma_start(
                        out=x_b[:, c8:c8 + 2, :, :],
                        in_=xb_d[:, c8 * 1024:(c8 + 2) * 1024])

                # k for this column group
                ps = psB.tile([128, 512], F32, tag="psb")
                nc.tensor.matmul(ps[:, :], lhsT=w_b[:, 0, 128:256],
                                 rhs=x_b[:, c8, 0, :], start=True, stop=False)
                nc.tensor.matmul(ps[:, :], lhsT=w_b[:, 1, 128:256],
                                 rhs=x_b[:, c8, 1, :], start=False, stop=True)
                nc.vector.tensor_scalar_add(k_sb[:, sl], ps[:, :], bk4[:, :])

                # q only for the own-half columns (0:2048 after rotation)
                if c8 < 4:
                    ps = psB.tile([128, 512], F32, tag="psb")
                    nc.tensor.matmul(ps[:, :], lhsT=w_b[:, 0, 0:128],
                                     rhs=x_b[:, c8, 0, :],
                                     start=True, stop=False)
                    nc.tensor.matmul(ps[:, :], lhsT=w_b[:, 1, 0:128],
                                     rhs=x_b[:, c8, 1, :],
                                     start=False, stop=True)
                    nc.vector.tensor_scalar_add(q_sb[:, sl], ps[:, :], bq4[:, :])

            def scores_block(b, e_blk):
                """Phase A: 2x row-packed K=32 score pairs + exp; pair g
                alternates PE row-tile quadrants so pairs can overlap."""
                i0 = b * 512
                for g in range(JT // 2):
                    ps = psA.tile([128, 2, 512], F32, tag="psa")
                    for t in range(2):
                        jt = 2 * g + t
                        tp = 32 * t + 64 * (g % 2)
                        nc.tensor.matmul(
                            ps[:, t, :],
                            lhsT=k_sb[tp:tp + 32,
                                      jt * 128:(jt + 1) * 128],
                            rhs=q_sb[tp:tp + 32, i0:i0 + 512],
                            start=True, stop=True,
                            tile_position=(tp, 0))
                    nc.scalar.activation(
                        e_blk[:, 2 * g:2 * g + 2, :], ps[:, :, :], act.Exp)

            def rowsum_block(b, e_blk):
                """Rowsum partials via column-tiled M=32 matmuls, then the
                fused fp32 combine+broadcast, reciprocal and gamma scale."""
                rs_ps = psA.tile([128, 512], F32, tag="psa", name="rs_ps")
                for t in range(4):
                    nc.tensor.matmul(
                        rs_ps[32 * t:32 * (t + 1), :], lhsT=zeros_rs[:, :],
                        rhs=e_blk[:, t, :], start=True, stop=False,
                        tile_position=(0, 32 * t), skip_group_check=True)
                for qd in range(NQUAD):
                    for t in range(4):
                        jt = 4 * qd + t
                        nc.tensor.matmul(
                            rs_ps[32 * t:32 * (t + 1), :],
                            lhsT=ones_rs[:, :],
                            rhs=e_blk[:, jt, :],
                            start=False, stop=(qd == NQUAD - 1),
                            tile_position=(0, 32 * t),
                            skip_group_check=True)
                # combine in two 256-col halves into separate PSUM tiles
                # (matmul outputs must be bank-aligned): the first combine
                # waits only a half-copy, the second overlaps the first
                rs_sb = smallp.tile([128, 512], F32, tag="rs")
                inv = smallp.tile([128, 512], F32, tag="inv")
                rg = smallp.tile([128, 512], F32, tag="rg")
                for hh in range(2):
                    es = slice(hh * 256, (hh + 1) * 256)
                    nc.vector.tensor_copy(rs_sb[:, es], rs_ps[:, es])
                    bc = psB.tile([128, 512], F32, tag="psb",
                                  name=f"bc_ps{hh}")
                    nc.tensor.matmul(bc[:, 0:256], lhsT=sel_w[:, :],
                                     rhs=rs_sb[:, es], start=True, stop=True)
                    nc.vector.reciprocal(inv[:, es], bc[:, 0:256])
                    nc.vector.tensor_scalar_mul(rg[:, es], inv[:, es],
                                                g128[:, :])
                return rg

            def out_block(b, e_blk, rg):
                """Phase B: out accumulation chains + epilogue + store."""
                i0 = b * 512
                for ch in range(2):
                    acc = psB.tile([128, 512], F32, tag="psb")
                    for jt in range(JT):
                        nc.tensor.matmul(
                            acc[:, :],
                            lhsT=v_sb[:, jt, ch * 128:(ch + 1) * 128],
                            rhs=e_blk[:, jt, :],
                            start=(jt == 0), stop=(jt == JT - 1))
                    res = resp.tile([128, 512], F32, tag="res")
                    for hh in range(2):
                        es = slice(hh * 256, (hh + 1) * 256)
                        nc.vector.tensor_mul(res[:, es], acc[:, es],
                                             rg[:, es])
                        nc.vector.scalar_tensor_tensor(
                            res[:, es], res[:, es], gb[:, ch:ch + 1],
                            x_b[:, i0 // 512, ch,
                                hh * 256:(hh + 1) * 256],
                            op0=mybir.AluOpType.add, op1=mybir.AluOpType.add)
                        nc.sync.dma_start(
                            out=out_d[ch * 128:(ch + 1) * 128,
                                      i0 + hh * 256:i0 + (hh + 1) * 256],
                            in_=res[:, es])

            # ---- software-pipelined attention blocks ----
            e_blks = [None, None]
            rgs = [None, None]

            e_blks[0] = eblkp.tile([128, JT, 512], BF16, tag="e", name="e_blk0")
            scores_block(0, e_blks[0])

            # v^T tiles: fill the pipeline prologue while exp(b0) runs
            for nt in range(JT):
                c8, r = divmod(nt, 4)
                ps = psB.tile([128, 512], F32, tag="psb")
                nc.tensor.matmul(
                    ps[:, 0:C],
                    lhsT=x_b[:, c8, 0, r * 128:(r + 1) * 128],
                    rhs=w_b[:, 0, 256:512], start=True, stop=False)
                nc.tensor.matmul(
                    ps[:, 0:C],
                    lhsT=x_b[:, c8, 1, r * 128:(r + 1) * 128],
                    rhs=w_b[:, 1, 256:512], start=False, stop=True)
                nc.vector.tensor_copy(v_sb[:, nt, :], ps[:, 0:C])

            rgs[0] = rowsum_block(0, e_blks[0])

            for b in range(1, NBLK):
                e_blks[b % 2] = eblkp.tile([128, JT, 512], BF16, tag="e", name=f"e_blk{b}")
                scores_block(b, e_blks[b % 2])
                out_block(b - 1, e_blks[(b - 1) % 2], rgs[(b - 1) % 2])
                rgs[b % 2] = rowsum_block(b, e_blks[b % 2])

            out_block(NBLK - 1, e_blks[(NBLK - 1) % 2], rgs[(NBLK - 1) % 2])

    return nc


_CACHE = {}


def _make_in_maps(x, Wq, bq, Wk, bk, Wv, bv, gamma):
    import ml_dtypes

    # host-side layout prep (relayout + bf16 rounding)
    wT = np.concatenate(
        [
            np.tile(np.ascontiguousarray(Wq.T), (1, 4)),
            np.tile(np.ascontiguousarray(Wk.T), (1, 4)),
            np.ascontiguousarray(Wv.T),
        ],
        axis=1,
    ).astype(ml_dtypes.bfloat16)              # [256, 512]
    bq4 = np.tile(bq, 4).reshape(128, 1)
    bk4 = np.tile(bk, 4).reshape(128, 1)
    bvP = np.ascontiguousarray(bv.reshape(2, 128).T)
    g128 = np.full((128, 1), float(gamma.reshape(-1)[0]))
    cst = np.concatenate([bq4, bk4, bvP, g128], axis=1).astype(np.float32)

    core_ids = list(range(8))
    in_maps = []
    for core in core_ids:
        b, half = divmod(core, 2)
        xf = x[b].reshape(C, N)
        # rotate so this core's own token-half sits in columns 0:NH
        # (one shared SPMD program; j-order is irrelevant to the softmax sum)
        if half == 1:
            xf = np.concatenate([xf[:, NH:], xf[:, :NH]], axis=1)
        # [128 part, 8 chunks, 2 cin-halves, 512 cols] contiguous
        xh = xf.reshape(2, 128, 8, 512).transpose(1, 2, 0, 3)
        xb = np.ascontiguousarray(xh).reshape(128, 2 * N)
        xb = xb.astype(ml_dtypes.bfloat16)
        in_maps.append(
            {
                "xb": xb,
                "wT": wT,
                "cst": cst,
            }
        )
    return in_maps


def kernel(x, Wq, bq, Wk, bk, Wv, bv, gamma):
    x = np.asarray(x, dtype=np.float32)
    Wq = np.asarray(Wq, dtype=np.float32)
    bq = np.asarray(bq, dtype=np.float32)
    Wk = np.asarray(Wk, dtype=np.float32)
    bk = np.asarray(bk, dtype=np.float32)
    Wv = np.asarray(Wv, dtype=np.float32)
    bv = np.asarray(bv, dtype=np.float32)
    gamma = np.asarray(gamma, dtype=np.float32)

    if "nc" not in _CACHE:
        _CACHE["nc"] = _build_program()
    nc = _CACHE["nc"]

    in_maps = _make_in_maps(x, Wq, bq, Wk, bk, Wv, bv, gamma)
    core_ids = list(range(8))

    from concourse.bass_utils import run_bass_kernel_spmd

    res = run_bass_kernel_spmd(nc, in_maps, core_ids)

    out = np.empty((B, C, N), dtype=np.float32)
    for core in core_ids:
        b, half = divmod(core, 2)
        out[b, :, half * NH:(half + 1) * NH] = res.results[core]["out"]
    return out.reshape(B, C, H, W)


# revision 33
# speedup vs baseline: 1.1015x; 1.1015x over previous
"""Position-attention (SAGAN-style) Bass kernel for 8 Trainium2 NeuronCores.

Reference computation (per batch b, with n = H*W = 4096 spatial tokens):
    q = Wq @ x + bq            [32, n]
    k = Wk @ x + bk            [32, n]
    v = Wv @ x + bv            [256, n]
    att = softmax_j(q_i . k_j) [n, n]
    out = gamma * (v @ att^T) + x

Sharding: 8 cores = 4 batches x 2 token-halves. Each core computes out for
its 2048-token half (the q/i side); k/v cover all 4096 tokens redundantly
(cheap). Output slices are disjoint -> no collectives. To keep one SPMD
program, the host rotates each core's x so its own half occupies columns
0:2048 (j-order is irrelevant to the softmax sum; k/v/e stay consistent).

Engine/schedule choices (all aimed at keeping the PE array dense):
  - x and the weights are pre-rounded to bf16 on the host and DMAed in a
    per-partition-contiguous layout; q/k projections interleave with the
    chunked x transfers so the PE starts before the load completes.
  - scores^T[j, i] (K=32) as 2x row-packed pairs into bank-aligned PSUM
    slices; consecutive pairs alternate PE row-tile quadrants
    (T0/T4 <-> T8/T12) so back-to-back pairs overlap (measured ~2x); exp
    on ACT reads each pair tile.
  - rowsum[i] = sum_j e^T[j, i] via column-tiled M=32 matmuls (partial
    sums replicated in partitions 32t:32t+32, accumulated over j-quads,
    measured ~4x concurrency). Four column-mode zero matmuls claim the
    bank first (matmul start=True invalidates has_written for the whole
    bank, so the four interleaved chains must accumulate with
    start=False onto an explicitly zero-filled bank). One fp32 matmul
    (lhsT = ones/32) then fuses combine + broadcast-to-128-partitions;
    reciprocal (DVE) and gamma give the per-i scale.
  - software pipeline per i-block b: scores(b) | out-chains(b-1) |
    rowsum(b) -- exp(b) hides under out(b-1); v-projection slots between
    scores(b0) and rowsum(b0) to fill the pipeline prologue.
  - out[c, i]: K=128 per j-tile, 32-matmul PSUM accumulation chains per
    (i-block, c-half); epilogue folds gamma/rowsum, gamma*bv and the
    bf16-rounded residual + x, split in halves to overlap the store DMA.
Matmul operands are bf16 (fp32 PSUM accumulation). PSUM budget: psA
3 bufs x 2 banks (score pairs, rowsum accumulator rides a psA slot) +
psB 2 bufs x 1 bank (v-proj/out chains, combine/broadcast) = 8 banks.
"""

import os
import sys

for _p in (
    "/root/.axon_site",
    "/root/.axon_site/_ro/trn_rl_repo",
    "/root/.axon_site/_ro/pypackages",
    "/opt/trn_rl_repo",
):
    if os.path.isdir(_p) and _p not in sys.path:
        sys.path.append(_p)

import json

import numpy as np

from concourse import bass, mybir
from concourse.tile import TileContext

F32 = mybir.dt.float32
BF16 = mybir.dt.bfloat16

B, C, H, W = 4, 256, 64, 64
N = H * W            # 4096 tokens
NH = N // 2          # 2048 tokens per core (token half)
MID = C // 8         # 32 qk channels
JT = N // 128        # 32 j-tiles of 128 tokens
NBLK = NH // 512     # 4 i-blocks of 512 tokens per core
NQUAD = JT // 4      # 8 quads of 4 j-tiles


def _split_multi_waits(bir_bytes: bytes) -> bytes:
    """Workaround for this container's walrus: it accepts at most ONE sem-wait
    command per lowered instruction ('Too many sync wait commands'), while
    bass/Tile freely attach several. Split extra waits onto preceding NoOps
    on the same engine — per-engine program order makes this semantics-
    preserving (all waits still satisfied before the instruction runs)."""
    d = json.loads(bir_bytes)
    n_split = 0
    for f in d.get("functions", []):
        for bb in f.get("blocks", []):
            out = []
            for ins in bb.get("instructions", []):
                si = ins.get("sync_info")
                waits = si.get("on_wait") if si else None
                if waits and len(waits) > 1:
                    for w in waits[:-1]:
                        n_split += 1
                        out.append(
                            {
                                "debug": ins.get("debug", 0),
                                "engine": ins["engine"],
                                "ins": [],
                                "outs": [],
                                "name": f"{ins['name']}-ws{n_split}",
                                "opcode": "NoOp",
                                "sync_info": {"on_wait": [w], "on_update": []},
                            }
                        )
                    si["on_wait"] = [waits[-1]]
                out.append(ins)
            bb["instructions"] = out
    return json.dumps(d).encode()


_ws_applied = False


def _apply_wait_split_patch():
    global _ws_applied
    if _ws_applied:
        return
    _ws_applied = True
    from concourse import bass_utils, bass2jax

    orig = bass_utils.compile_bir_kernel

    def patched(bir_json, tmpdir, neff_name="file.neff"):
        return orig(_split_multi_waits(bytes(bir_json)), tmpdir, neff_name)

    bass_utils.compile_bir_kernel = patched
    bass2jax.compile_bir_kernel = patched


_apply_wait_split_patch()


def _build_program():
    nc = bass.Bass()

    xb_d = nc.declare_dram_parameter("xb", [128, 2 * N], BF16, isOutput=False)
    wT_d = nc.declare_dram_parameter("wT", [C, 512], BF16, isOutput=False)
    cst_d = nc.declare_dram_parameter("cst", [128, 5], F32, isOutput=False)
    out_d = nc.declare_dram_parameter("out", [C, NH], F32, isOutput=True)

    act = mybir.ActivationFunctionType

    with TileContext(nc) as tc:
        with (
            tc.tile_pool(name="const", bufs=1) as constp,
            tc.tile_pool(name="xb", bufs=1) as xbp,
            tc.tile_pool(name="proj", bufs=1) as projp,
            tc.tile_pool(name="eblk", bufs=2) as eblkp,
            tc.tile_pool(name="small", bufs=4) as smallp,
            tc.tile_pool(name="res", bufs=4) as resp,
            tc.tile_pool(name="psA", bufs=3, space="PSUM") as psA,
            tc.tile_pool(name="psB", bufs=2, space="PSUM") as psB,
        ):
            # ---- constants / weights ----
            w_b = constp.tile([128, 2, 512], BF16, tag="wb")
            for h in range(2):
                nc.sync.dma_start(out=w_b[:, h, :],
                                  in_=wT_d[h * 128:(h + 1) * 128, :])

            cst = constp.tile([128, 5], F32, tag="cst")
            nc.sync.dma_start(out=cst[:, :], in_=cst_d[:, :])
            bq4 = cst[:, 0:1]
            bk4 = cst[:, 1:2]
            bvP = cst[:, 2:4]
            g128 = cst[:, 4:5]
            # gb[c] = gamma * bv[c]  (folded v-bias: out += gamma*bv[c])
            gb = constp.tile([128, 2], F32, tag="gb")
            nc.vector.tensor_scalar_mul(gb[:, :], bvP[:, :], g128[:, :])

            # ones[128, 32] bf16 for the column-tiled rowsum matmuls
            ones_rs = constp.tile([128, 32], BF16, tag="ones_rs")
            nc.vector.memset(ones_rs[:, :], 1.0)
            # zeros for the column-mode matmuls that claim the rowsum bank
            zeros_rs = constp.tile([128, 32], BF16, tag="zeros_rs")
            nc.vector.memset(zeros_rs[:, :], 0.0)
            # combine+broadcast weights: rowsum lands 32x-replicated per
            # column group, so sum over all 128 partitions / 32 = rowsum
            sel_w = constp.tile([128, 128], F32, tag="sel_w")
            nc.vector.memset(sel_w[:, :], 1.0 / 32.0)

            # HAM warm-up: ~4us of dummy matmuls while the x DMA is in
            # flight, so the PE clock-gate is already at 8/8 (2.4 GHz)
            # when the first projection matmuls issue. Results are never
            # read; the psA slot is recycled by the scores pipeline.
            warm_rhs = constp.tile([128, 512], BF16, tag="warm_rhs")
            nc.vector.memset(warm_rhs[:, :], 0.0)
            warm_ps = psA.tile([128, 512], F32, tag="psa", name="warm_ps")
            for _ in range(10):
                nc.tensor.matmul(warm_ps[0:32, :], lhsT=zeros_rs[:, :],
                                 rhs=warm_rhs[:, :], start=True, stop=True,
                                 skip_group_check=True)

            # ---- x load (chunked, SP ring, bf16) + q/k projections ----
            # host layout [128, c8, h, 512]: per-partition contiguous so
            # each chunk DMA is one descriptor per partition
            x_b = xbp.tile([128, 8, 2, 512], BF16, tag="xb")
            q_sb = projp.tile([128, NH], BF16, tag="q")
            k_sb = projp.tile([128, N], BF16, tag="k")
            v_sb = projp.tile([128, JT, C], BF16, tag="v")

            for c8 in range(8):
                sl = slice(c8 * 512, (c8 + 1) * 512)
                if c8 % 2 == 0:
                    nc.sync.dma_start(
                        out=x_b[:, c8:c8 + 2, :, :],
                        in_=xb_d[:, c8 * 1024:(c8 + 2) * 1024])

                # k for this column group
                ps = psB.tile([128, 512], F32, tag="psb")
                nc.tensor.matmul(ps[:, :], lhsT=w_b[:, 0, 128:256],
                                 rhs=x_b[:, c8, 0, :], start=True, stop=False)
                nc.tensor.matmul(ps[:, :], lhsT=w_b[:, 1, 128:256],
                                 rhs=x_b[:, c8, 1, :], start=False, stop=True)
                nc.vector.tensor_scalar_add(k_sb[:, sl], ps[:, :], bk4[:, :])

                # q only for the own-half columns (0:2048 after rotation)
                if c8 < 4:
                    ps = psB.tile([128, 512], F32, tag="psb")
                    nc.tensor.matmul(ps[:, :], lhsT=w_b[:, 0, 0:128],
                                     rhs=x_b[:, c8, 0, :],
                                     start=True, stop=False)
                    nc.tensor.matmul(ps[:, :], lhsT=w_b[:, 1, 0:128],
                                     rhs=x_b[:, c8, 1, :],
                                     start=False, stop=True)
                    nc.vector.tensor_scalar_add(q_sb[:, sl], ps[:, :], bq4[:, :])

            def scores_block(b, e_blk):
                """Phase A: 2x row-packed K=32 score pairs + exp; pair g
                alternates PE row-tile quadrants so pairs can overlap."""
                i0 = b * 512
                for g in range(JT // 2):
                    ps = psA.tile([128, 2, 512], F32, tag="psa")
                    for t in range(2):
                        jt = 2 * g + t
                        tp = 32 * t + 64 * (g % 2)
                        nc.tensor.matmul(
                            ps[:, t, :],
                            lhsT=k_sb[tp:tp + 32,
                                      jt * 128:(jt + 1) * 128],
                            rhs=q_sb[tp:tp + 32, i0:i0 + 512],
                            start=True, stop=True,
                            tile_position=(tp, 0))
                    nc.scalar.activation(
                        e_blk[:, 2 * g:2 * g + 2, :], ps[:, :, :], act.Exp)

            def rowsum_block(b, e_blk):
                """Rowsum partials via column-tiled M=32 matmuls, then the
                fused fp32 combine+broadcast, reciprocal and gamma scale."""
                rs_ps = psA.tile([128, 512], F32, tag="psa", name="rs_ps")
                for t in range(4):
                    nc.tensor.matmul(
                        rs_ps[32 * t:32 * (t + 1), :], lhsT=zeros_rs[:, :],
                        rhs=e_blk[:, t, :], start=True, stop=False,
                        tile_position=(0, 32 * t), skip_group_check=True)
                for qd in range(NQUAD):
                    for t in range(4):
                        jt = 4 * qd + t
                        nc.tensor.matmul(
                            rs_ps[32 * t:32 * (t + 1), :],
                            lhsT=ones_rs[:, :],
                            rhs=e_blk[:, jt, :],
                            start=False, stop=(qd == NQUAD - 1),
                            tile_position=(0, 32 * t),
                            skip_group_check=True)
                rs_sb = smallp.tile([128, 512], F32, tag="rs")
                nc.vector.tensor_copy(rs_sb[:, :], rs_ps[:, :])
                bc_ps = psB.tile([128, 512], F32, tag="psb", name="bc_ps")
                nc.tensor.matmul(bc_ps[:, :], lhsT=sel_w[:, :],
                                 rhs=rs_sb[:, :], start=True, stop=True)
                inv = smallp.tile([128, 512], F32, tag="inv")
                nc.vector.reciprocal(inv[:, :], bc_ps[:, :])
                rg = smallp.tile([128, 512], F32, tag="rg")
                nc.vector.tensor_scalar_mul(rg[:, :], inv[:, :], g128[:, :])
                return rg

            def out_block(b, e_blk, rg):
                """Phase B: out accumulation chains + epilogue + store."""
                i0 = b * 512
                for ch in range(2):
                    acc = psB.tile([128, 512], F32, tag="psb")
                    for jt in range(JT):
                        nc.tensor.matmul(
                            acc[:, :],
                            lhsT=v_sb[:, jt, ch * 128:(ch + 1) * 128],
                            rhs=e_blk[:, jt, :],
                            start=(jt == 0), stop=(jt == JT - 1))
                    res = resp.tile([128, 512], F32, tag="res")
                    for hh in range(2):
                        es = slice(hh * 256, (hh + 1) * 256)
                        nc.vector.tensor_mul(res[:, es], acc[:, es],
                                             rg[:, es])
                        nc.vector.scalar_tensor_tensor(
                            res[:, es], res[:, es], gb[:, ch:ch + 1],
                            x_b[:, i0 // 512, ch,
                                hh * 256:(hh + 1) * 256],
                            op0=mybir.AluOpType.add, op1=mybir.AluOpType.add)
                        nc.sync.dma_start(
                            out=out_d[ch * 128:(ch + 1) * 128,
                                      i0 + hh * 256:i0 + (hh + 1) * 256],
                            in_=res[:, es])

            # ---- software-pipelined attention blocks ----
            e_blks = [None, None]
            rgs = [None, None]

            e_blks[0] = eblkp.tile([128, JT, 512], BF16, tag="e", name="e_blk0")
            scores_block(0, e_blks[0])

            # v^T tiles: fill the pipeline prologue while exp(b0) runs
            for nt in range(JT):
                c8, r = divmod(nt, 4)
                ps = psB.tile([128, 512], F32, tag="psb")
                nc.tensor.matmul(
                    ps[:, 0:C],
                    lhsT=x_b[:, c8, 0, r * 128:(r + 1) * 128],
                    rhs=w_b[:, 0, 256:512], start=True, stop=False)
                nc.tensor.matmul(
                    ps[:, 0:C],
                    lhsT=x_b[:, c8, 1, r * 128:(r + 1) * 128],
                    rhs=w_b[:, 1, 256:512], start=False, stop=True)
                nc.vector.tensor_copy(v_sb[:, nt, :], ps[:, 0:C])

            rgs[0] = rowsum_block(0, e_blks[0])

            for b in range(1, NBLK):
                e_blks[b % 2] = eblkp.tile([128, JT, 512], BF16, tag="e", name=f"e_blk{b}")
                scores_block(b, e_blks[b % 2])
                out_block(b - 1, e_blks[(b - 1) % 2], rgs[(b - 1) % 2])
                rgs[b % 2] = rowsum_block(b, e_blks[b % 2])

            out_block(NBLK - 1, e_blks[(NBLK - 1) % 2], rgs[(NBLK - 1) % 2])

    return nc


_CACHE = {}


def _make_in_maps(x, Wq, bq, Wk, bk, Wv, bv, gamma):
    import ml_dtypes

    # host-side layout prep (relayout + bf16 rounding)
    wT = np.concatenate(
        [
            np.tile(np.ascontiguousarray(Wq.T), (1, 4)),
            np.tile(np.ascontiguousarray(Wk.T), (1, 4)),
            np.ascontiguousarray(Wv.T),
        ],
        axis=1,
    ).astype(ml_dtypes.bfloat16)              # [256, 512]
    bq4 = np.tile(bq, 4).reshape(128, 1)
    bk4 = np.tile(bk, 4).reshape(128, 1)
    bvP = np.ascontiguousarray(bv.reshape(2, 128).T)
    g128 = np.full((128, 1), float(gamma.reshape(-1)[0]))
    cst = np.concatenate([bq4, bk4, bvP, g128], axis=1).astype(np.float32)

    core_ids = list(range(8))
    in_maps = []
    for core in core_ids:
        b, half = divmod(core, 2)
        xf = x[b].reshape(C, N)
        # rotate so this core's own token-half sits in columns 0:NH
        # (one shared SPMD program; j-order is irrelevant to the softmax sum)
        if half == 1:
            xf = np.concatenate([xf[:, NH:], xf[:, :NH]], axis=1)
        # [128 part, 8 chunks, 2 cin-halves, 512 cols] contiguous
        xh = xf.reshape(2, 128, 8, 512).transpose(1, 2, 0, 3)
        xb = np.ascontiguousarray(xh).reshape(128, 2 * N)
        xb = xb.astype(ml_dtypes.bfloat16)
        in_maps.append(
            {
                "xb": xb,
                "wT": wT,
                "cst": cst,
            }
        )
    return in_maps


def kernel(x, Wq, bq, Wk, bk, Wv, bv, gamma):
    x = np.asarray(x, dtype=np.float32)
    Wq = np.asarray(Wq, dtype=np.float32)
    bq = np.asarray(bq, dtype=np.float32)
    Wk = np.asarray(Wk, dtype=np.float32)
    bk = np.asarray(bk, dtype=np.float32)
    Wv = np.asarray(Wv, dtype=np.float32)
    bv = np.asarray(bv, dtype=np.float32)
    gamma = np.asarray(gamma, dtype=np.float32)

    if "nc" not in _CACHE:
        _CACHE["nc"] = _build_program()
    nc = _CACHE["nc"]

    in_maps = _make_in_maps(x, Wq, bq, Wk, bk, Wv, bv, gamma)
    core_ids = list(range(8))

    from concourse.bass_utils import run_bass_kernel_spmd

    res = run_bass_kernel_spmd(nc, in_maps, core_ids)

    out = np.empty((B, C, N), dtype=np.float32)
    for core in core_ids:
        b, half = divmod(core, 2)
        out[b, :, half * NH:(half + 1) * NH] = res.results[core]["out"]
    return out.reshape(B, C, H, W)
